# revision 7
# baseline (speedup 1.0000x reference)
"""Trainium2 Bass kernel for batched cosine similarity (retrieval_knn).

sim[s, b] = dot(support[s,b,:], X[b,:]) / (max(||support[s,b]||, eps) * max(||X[b]||, eps))
optionally normalized to (sim + 1) / 2.

Shapes: support [512, 4096, 64] f32, X [4096, 64] f32 -> out [512, 4096] f32.

Strategy (8 NeuronCores, data-parallel over the batch axis):
  - Each core handles a contiguous 512-wide slice of b. 64 MB of support
    data per core; memory-bound problem.
  - Host folds 1/max(||X_b||, eps) into X (Xn), packs Xn into zero-padded
    bf16 matmul weights, so the device only computes:
        dot_n[s,b] = sum_d A[s,b,d] * Xn[b,d]     (TensorE)
        sqn[s,b]   = sum_d A[s,b,d]^2             (TensorE, on squared tiles)
        sim        = dot_n * rsqrt-ish(sqn)       (Scalar/Vector engines)
  - Support is cast fp32->bf16 during the DMA load (SWDGE cast).
  - Natural [s-partition, (b,d)-free] tiles are transposed on TensorE
    (128x128 blocks) into [(b-pair, d)-partition, s-free] layout so the
    d-contraction lands on the partition axis where the PE can reduce it.
  - Per pair of b's: one dot matmul and one squared matmul with N=S,
    accumulated per 32-partition col-group stripe of a PSUM bank.
"""

import numpy as np
import ml_dtypes

BF16 = ml_dtypes.bfloat16

S, B, D = 512, 4096, 64
NCORES = 8
BL = B // NCORES  # 512 batch elements per core

# fraction of PSUM->SBUF evacuation copies routed to ScalarE (rest on VectorE)
_EVAC_MOD = 3  # jq % 3 == 0 -> VectorE, else ScalarE

_prog_cache = {}


def _build(s_sz, bl_sz, normalize):
    from concourse import bacc, mybir
    from concourse.tile import TileContext
    from contextlib import ExitStack

    SBn = s_sz // 128   # number of 128-row s blocks
    Q = bl_sz // 128    # number of 128-wide b quads
    NP = bl_sz // 2     # number of b pairs

    nc = bacc.Bacc("TRN2")
    sup = nc.declare_dram_parameter("support", [s_sz, bl_sz, D], mybir.dt.float32, isOutput=False)
    xw = nc.declare_dram_parameter("xw", [128, NP * 32], mybir.dt.bfloat16, isOutput=False)
    onesw = nc.declare_dram_parameter("onesw", [128, 16 * 32], mybir.dt.bfloat16, isOutput=False)
    idb = nc.declare_dram_parameter("ident_bf", [128, 128], mybir.dt.bfloat16, isOutput=False)
    idf = nc.declare_dram_parameter("ident_f32", [128, 128], mybir.dt.float32, isOutput=False)
    out = nc.declare_dram_parameter("out", [s_sz, bl_sz], mybir.dt.float32, isOutput=True)

    with TileContext(nc) as tc, ExitStack() as ctx:
        singles = ctx.enter_context(tc.tile_pool(name="singles", bufs=1))
        natp = ctx.enter_context(tc.tile_pool(name="nat", bufs=16))
        tevp = ctx.enter_context(tc.tile_pool(name="tev", bufs=4))
        sqp = ctx.enter_context(tc.tile_pool(name="sqt", bufs=4))
        finp = ctx.enter_context(tc.tile_pool(name="fin", bufs=2))
        psT = ctx.enter_context(tc.tile_pool(name="psT", bufs=3, space="PSUM"))
        psDot = ctx.enter_context(tc.tile_pool(name="psDot", bufs=2, space="PSUM"))
        psSqn = ctx.enter_context(tc.tile_pool(name="psSqn", bufs=1, space="PSUM"))
        psOut = ctx.enter_context(tc.tile_pool(name="psOut", bufs=2, space="PSUM"))

        t_idb = singles.tile([128, 128], mybir.dt.bfloat16)
        nc.sync.dma_start(out=t_idb, in_=idb[:, :])
        t_idf = singles.tile([128, 128], mybir.dt.float32)
        nc.sync.dma_start(out=t_idf, in_=idf[:, :])
        t_xw = singles.tile([128, NP * 32], mybir.dt.bfloat16)
        nc.sync.dma_start(out=t_xw, in_=xw[:, :])
        t_ones = singles.tile([128, 16 * 32], mybir.dt.bfloat16)
        nc.sync.dma_start(out=t_ones, in_=onesw[:, :])
        out_stage = [
            singles.tile([128, bl_sz], mybir.dt.float32, name=f"ostage{sb}", tag=f"ostage{sb}")
            for sb in range(SBn)
        ]

        for q in range(Q):
            nat = {}
            for h in range(2):
                for sb in range(SBn):
                    t = natp.tile([128, 64 * D], mybir.dt.bfloat16, tag="nat")
                    nc.gpsimd.dma_start(
                        out=t,
                        in_=sup[sb * 128:(sb + 1) * 128,
                                q * 128 + h * 64: q * 128 + (h + 1) * 64, :],
                    )
                    nat[(h, sb)] = t

            dot_ps = psDot.tile([128, s_sz], mybir.dt.float32)
            sqn_ps = psSqn.tile([128, s_sz], mybir.dt.float32)

            for jq in range(64):
                c, l = jq // 16, jq % 16
                h, bh = c // 2, 32 * (c % 2) + 2 * l
                T_ps = psT.tile([128, s_sz], mybir.dt.bfloat16)
                for sb in range(SBn):
                    nc.tensor.transpose(
                        T_ps[:, sb * 128:(sb + 1) * 128],
                        nat[(h, sb)][:, bh * D:(bh + 2) * D],
                        t_idb,
                    )
                Tt = tevp.tile([128, s_sz], mybir.dt.bfloat16, tag="tev")
                if jq % _EVAC_MOD == 0:
                    nc.vector.tensor_copy(Tt, T_ps)
                else:
                    nc.scalar.copy(Tt, T_ps)
                Sq = sqp.tile([128, s_sz], mybir.dt.bfloat16, tag="sqt")
                nc.vector.tensor_mul(Sq, Tt, Tt)
                jp = q * 64 + jq
                nc.tensor.matmul(
                    dot_ps[32 * c:32 * (c + 1), :],
                    lhsT=t_xw[:, jp * 32:(jp + 1) * 32],
                    rhs=Tt,
                    start=(l == 0),
                    stop=(l == 15),
                    tile_position=(0, 32 * c),
                )
                nc.tensor.matmul(
                    sqn_ps[32 * c:32 * (c + 1), :],
                    lhsT=t_ones[:, l * 32:(l + 1) * 32],
                    rhs=Sq,
                    start=(l == 0),
                    stop=(l == 15),
                    tile_position=(0, 32 * c),
                )

            # finalize this quad: sim = dot * 1/max(sqrt(sqn), eps) (+ affine)
            sqv = finp.tile([128, s_sz], mybir.dt.float32, tag="fsq")
            nc.scalar.sqrt(sqv, sqn_ps)
            nc.vector.tensor_scalar_max(sqv, sqv, 1e-10)
            rv = finp.tile([128, s_sz], mybir.dt.float32, tag="frv")
            nc.vector.reciprocal(rv, sqv)
            simv = finp.tile([128, s_sz], mybir.dt.float32, tag="fsim")
            nc.vector.tensor_mul(simv, dot_ps, rv)
            if normalize:
                nc.vector.tensor_scalar(
                    simv, simv, 0.5, 0.5, mybir.AluOpType.mult, mybir.AluOpType.add
                )
            for sb in range(SBn):
                oT = psOut.tile([128, 128], mybir.dt.float32)
                nc.tensor.transpose(oT, simv[:, sb * 128:(sb + 1) * 128], t_idf)
                nc.vector.tensor_copy(out_stage[sb][:, q * 128:(q + 1) * 128], oT)

        for sb in range(SBn):
            nc.sync.dma_start(out=out[sb * 128:(sb + 1) * 128, :], in_=out_stage[sb])

    nc.finalize()
    return nc


def _pack_host_inputs(x_hat, bl_sz):
    """Fold 1/max(||x||,eps) into X, pack per-core zero-padded bf16 lhsT mats.

    Returns (xw_per_core list of [128, (bl//2)*32] bf16, onesw [128, 512] bf16).
    Pair jp (within a core) covers b_local = q*128 + 32*c + 2*l (+1), where
    q = jp // 64, c = (jp % 64) // 16, l = jp % 16.  lhsT column 2*l holds
    Xn[b_even] in partitions 0:64, column 2*l+1 holds Xn[b_odd] in 64:128.
    """
    x = np.asarray(x_hat, np.float32)
    xnorm = np.sqrt((x * x).sum(axis=1, keepdims=True))
    xn = (x / np.maximum(xnorm, 1e-10)).astype(BF16)

    ncores = x.shape[0] // bl_sz
    np_pairs = bl_sz // 2
    xw_cores = []
    for k in range(ncores):
        xw = np.zeros((128, np_pairs * 32), dtype=BF16)
        for jp in range(np_pairs):
            q, jq = jp // 64, jp % 64
            c, l = jq // 16, jq % 16
            b0 = k * bl_sz + q * 128 + 32 * c + 2 * l
            col = jp * 32
            xw[0:64, col + 2 * l] = xn[b0]
            xw[64:128, col + 2 * l + 1] = xn[b0 + 1]
        xw_cores.append(xw)

    onesw = np.zeros((128, 16 * 32), dtype=BF16)
    for l in range(16):
        onesw[0:64, l * 32 + 2 * l] = BF16(1.0)
        onesw[64:128, l * 32 + 2 * l + 1] = BF16(1.0)
    return xw_cores, onesw


def _get_program(normalize):
    key = (S, BL, bool(normalize))
    if key not in _prog_cache:
        _prog_cache[key] = _build(S, BL, bool(normalize))
    return _prog_cache[key]


def _run(support_set, X_hat, normalize, **spmd_kwargs):
    support_set = np.asarray(support_set)
    X_hat = np.asarray(X_hat, np.float32)
    nrm = bool(np.asarray(normalize).item())

    from concourse.bass_utils import run_bass_kernel_spmd

    nc = _get_program(nrm)
    xw_cores, onesw = _pack_host_inputs(X_hat, BL)
    ident_bf = np.eye(128, dtype=BF16)
    ident_f32 = np.eye(128, dtype=np.float32)

    in_maps = []
    for k in range(NCORES):
        shard = np.ascontiguousarray(support_set[:, k * BL:(k + 1) * BL, :], dtype=np.float32)
        in_maps.append({
            "support": shard,
            "xw": xw_cores[k],
            "onesw": onesw,
            "ident_bf": ident_bf,
            "ident_f32": ident_f32,
        })

    res = run_bass_kernel_spmd(nc, in_maps, list(range(NCORES)), **spmd_kwargs)
    out = np.concatenate(
        [np.asarray(res.results[k]["out"]) for k in range(NCORES)], axis=1
    )
    return np.ascontiguousarray(out, dtype=np.float32), res


def kernel(support_set, X_hat, normalize):
    out, _ = _run(support_set, X_hat, normalize)
    return out


# revision 9
# speedup vs baseline: 56.9463x; 56.9463x over previous
"""Trainium2 Bass kernel for batched cosine similarity (retrieval_knn).

sim[s, b] = dot(support[s,b,:], X[b,:]) / (max(||support[s,b]||, eps) * max(||X[b]||, eps))
optionally normalized to (sim + 1) / 2.

Shapes: support [512, 4096, 64] f32, X [4096, 64] f32 -> out [512, 4096] f32.

Strategy (8 NeuronCores, data-parallel over the batch axis):
  - Each core handles a contiguous 512-wide slice of b. 64 MB of support
    data per core; memory-bound problem.
  - Host folds 1/max(||X_b||, eps) into X (Xn), packs Xn into zero-padded
    bf16 matmul weights, so the device only computes:
        dot_n[s,b] = sum_d A[s,b,d] * Xn[b,d]     (TensorE)
        sqn[s,b]   = sum_d A[s,b,d]^2             (TensorE, on squared tiles)
        sim        = dot_n * rsqrt-ish(sqn)       (Scalar/Vector engines)
  - Support is cast fp32->bf16 during the DMA load (SWDGE cast).
  - Natural [s-partition, (b,d)-free] tiles are transposed on TensorE
    (128x128 blocks) into [(b-pair, d)-partition, s-free] layout so the
    d-contraction lands on the partition axis where the PE can reduce it.
  - Per pair of b's: one dot matmul and one squared matmul with N=S,
    accumulated per 32-partition col-group stripe of a PSUM bank.
"""

import numpy as np
import ml_dtypes

BF16 = ml_dtypes.bfloat16

S, B, D = 512, 4096, 64
NCORES = 8
BL = B // NCORES  # 512 batch elements per core

# fraction of PSUM->SBUF evacuation copies routed to ScalarE (rest on VectorE)
_EVAC_MOD = 3  # jq % 3 == 0 -> VectorE, else ScalarE

_prog_cache = {}


def _build(s_sz, bl_sz, normalize, loop_iters=1):
    from concourse import bacc, mybir
    from concourse.tile import TileContext
    from contextlib import ExitStack, nullcontext

    SBn = s_sz // 128   # number of 128-row s blocks
    Q = bl_sz // 128    # number of 128-wide b quads
    NP = bl_sz // 2     # number of b pairs

    nc = bacc.Bacc("TRN2")
    sup = nc.declare_dram_parameter("support", [s_sz, bl_sz, D], mybir.dt.float32, isOutput=False)
    xw = nc.declare_dram_parameter("xw", [128, NP * 32], mybir.dt.bfloat16, isOutput=False)
    onesw = nc.declare_dram_parameter("onesw", [128, 16 * 32], mybir.dt.bfloat16, isOutput=False)
    idb = nc.declare_dram_parameter("ident_bf", [128, 128], mybir.dt.bfloat16, isOutput=False)
    idf = nc.declare_dram_parameter("ident_f32", [128, 128], mybir.dt.float32, isOutput=False)
    out = nc.declare_dram_parameter("out", [s_sz, bl_sz], mybir.dt.float32, isOutput=True)

    with TileContext(nc) as tc, ExitStack() as ctx:
        singles = ctx.enter_context(tc.tile_pool(name="singles", bufs=1))
        natp = ctx.enter_context(tc.tile_pool(name="nat", bufs=16))
        tevp = ctx.enter_context(tc.tile_pool(name="tev", bufs=4))
        sqp = ctx.enter_context(tc.tile_pool(name="sqt", bufs=4))
        finp = ctx.enter_context(tc.tile_pool(name="fin", bufs=2))
        psT = ctx.enter_context(tc.tile_pool(name="psT", bufs=3, space="PSUM"))
        psDot = ctx.enter_context(tc.tile_pool(name="psDot", bufs=2, space="PSUM"))
        psSqn = ctx.enter_context(tc.tile_pool(name="psSqn", bufs=1, space="PSUM"))
        psOut = ctx.enter_context(tc.tile_pool(name="psOut", bufs=2, space="PSUM"))

        t_idb = singles.tile([128, 128], mybir.dt.bfloat16)
        nc.sync.dma_start(out=t_idb, in_=idb[:, :])
        t_idf = singles.tile([128, 128], mybir.dt.float32)
        nc.sync.dma_start(out=t_idf, in_=idf[:, :])
        t_xw = singles.tile([128, NP * 32], mybir.dt.bfloat16)
        nc.sync.dma_start(out=t_xw, in_=xw[:, :])
        t_ones = singles.tile([128, 16 * 32], mybir.dt.bfloat16)
        nc.sync.dma_start(out=t_ones, in_=onesw[:, :])
        out_stage = [
            singles.tile([128, bl_sz], mybir.dt.float32, name=f"ostage{sb}", tag=f"ostage{sb}")
            for sb in range(SBn)
        ]

        loop_ctx = tc.For_i(0, loop_iters, 1) if loop_iters > 1 else nullcontext()
        ctx.enter_context(loop_ctx)

        for q in range(Q):
            nat = {}
            for h in range(2):
                for sb in range(SBn):
                    t = natp.tile([128, 64 * D], mybir.dt.bfloat16, tag="nat")
                    nc.gpsimd.dma_start(
                        out=t,
                        in_=sup[sb * 128:(sb + 1) * 128,
                                q * 128 + h * 64: q * 128 + (h + 1) * 64, :],
                    )
                    nat[(h, sb)] = t

            dot_ps = psDot.tile([128, s_sz], mybir.dt.float32)
            sqn_ps = psSqn.tile([128, s_sz], mybir.dt.float32)

            for jq in range(64):
                c, l = jq // 16, jq % 16
                h, bh = c // 2, 32 * (c % 2) + 2 * l
                T_ps = psT.tile([128, s_sz], mybir.dt.bfloat16)
                for sb in range(SBn):
                    nc.tensor.transpose(
                        T_ps[:, sb * 128:(sb + 1) * 128],
                        nat[(h, sb)][:, bh * D:(bh + 2) * D],
                        t_idb,
                    )
                Tt = tevp.tile([128, s_sz], mybir.dt.bfloat16, tag="tev")
                if jq % _EVAC_MOD == 0:
                    nc.vector.tensor_copy(Tt, T_ps)
                else:
                    nc.scalar.copy(Tt, T_ps)
                Sq = sqp.tile([128, s_sz], mybir.dt.bfloat16, tag="sqt")
                nc.vector.tensor_mul(Sq, Tt, Tt)
                jp = q * 64 + jq
                nc.tensor.matmul(
                    dot_ps[32 * c:32 * (c + 1), :],
                    lhsT=t_xw[:, jp * 32:(jp + 1) * 32],
                    rhs=Tt,
                    start=(l == 0),
                    stop=(l == 15),
                    tile_position=(0, 32 * c),
                )
                nc.tensor.matmul(
                    sqn_ps[32 * c:32 * (c + 1), :],
                    lhsT=t_ones[:, l * 32:(l + 1) * 32],
                    rhs=Sq,
                    start=(l == 0),
                    stop=(l == 15),
                    tile_position=(0, 32 * c),
                )

            # finalize this quad: sim = dot * 1/max(sqrt(sqn), eps) (+ affine)
            sqv = finp.tile([128, s_sz], mybir.dt.float32, tag="fsq")
            nc.scalar.sqrt(sqv, sqn_ps)
            nc.vector.tensor_scalar_max(sqv, sqv, 1e-10)
            rv = finp.tile([128, s_sz], mybir.dt.float32, tag="frv")
            nc.vector.reciprocal(rv, sqv)
            simv = finp.tile([128, s_sz], mybir.dt.float32, tag="fsim")
            nc.vector.tensor_mul(simv, dot_ps, rv)
            if normalize:
                nc.vector.tensor_scalar(
                    simv, simv, 0.5, 0.5, mybir.AluOpType.mult, mybir.AluOpType.add
                )
            for sb in range(SBn):
                oT = psOut.tile([128, 128], mybir.dt.float32)
                nc.tensor.transpose(oT, simv[:, sb * 128:(sb + 1) * 128], t_idf)
                nc.vector.tensor_copy(out_stage[sb][:, q * 128:(q + 1) * 128], oT)

        for sb in range(SBn):
            nc.sync.dma_start(out=out[sb * 128:(sb + 1) * 128, :], in_=out_stage[sb])

    nc.finalize()
    return nc


def _pack_host_inputs(x_hat, bl_sz):
    """Fold 1/max(||x||,eps) into X, pack per-core zero-padded bf16 lhsT mats.

    Returns (xw_per_core list of [128, (bl//2)*32] bf16, onesw [128, 512] bf16).
    Pair jp (within a core) covers b_local = q*128 + 32*c + 2*l (+1), where
    q = jp // 64, c = (jp % 64) // 16, l = jp % 16.  lhsT column 2*l holds
    Xn[b_even] in partitions 0:64, column 2*l+1 holds Xn[b_odd] in 64:128.
    """
    x = np.asarray(x_hat, np.float32)
    xnorm = np.sqrt((x * x).sum(axis=1, keepdims=True))
    xn = (x / np.maximum(xnorm, 1e-10)).astype(BF16)

    ncores = x.shape[0] // bl_sz
    np_pairs = bl_sz // 2
    xw_cores = []
    for k in range(ncores):
        xw = np.zeros((128, np_pairs * 32), dtype=BF16)
        for jp in range(np_pairs):
            q, jq = jp // 64, jp % 64
            c, l = jq // 16, jq % 16
            b0 = k * bl_sz + q * 128 + 32 * c + 2 * l
            col = jp * 32
            xw[0:64, col + 2 * l] = xn[b0]
            xw[64:128, col + 2 * l + 1] = xn[b0 + 1]
        xw_cores.append(xw)

    onesw = np.zeros((128, 16 * 32), dtype=BF16)
    for l in range(16):
        onesw[0:64, l * 32 + 2 * l] = BF16(1.0)
        onesw[64:128, l * 32 + 2 * l + 1] = BF16(1.0)
    return xw_cores, onesw


def _get_program(normalize):
    key = (S, BL, bool(normalize))
    if key not in _prog_cache:
        _prog_cache[key] = _build(S, BL, bool(normalize))
    return _prog_cache[key]


def _run(support_set, X_hat, normalize, **spmd_kwargs):
    support_set = np.asarray(support_set)
    X_hat = np.asarray(X_hat, np.float32)
    nrm = bool(np.asarray(normalize).item())

    from concourse.bass_utils import run_bass_kernel_spmd

    nc = _get_program(nrm)
    xw_cores, onesw = _pack_host_inputs(X_hat, BL)
    ident_bf = np.eye(128, dtype=BF16)
    ident_f32 = np.eye(128, dtype=np.float32)

    in_maps = []
    for k in range(NCORES):
        shard = np.ascontiguousarray(support_set[:, k * BL:(k + 1) * BL, :], dtype=np.float32)
        in_maps.append({
            "support": shard,
            "xw": xw_cores[k],
            "onesw": onesw,
            "ident_bf": ident_bf,
            "ident_f32": ident_f32,
        })

    res = run_bass_kernel_spmd(nc, in_maps, list(range(NCORES)), **spmd_kwargs)
    out = np.concatenate(
        [np.asarray(res.results[k]["out"]) for k in range(NCORES)], axis=1
    )
    return np.ascontiguousarray(out, dtype=np.float32), res


def kernel(support_set, X_hat, normalize):
    out, _ = _run(support_set, X_hat, normalize)
    return out


# revision 17
# speedup vs baseline: 74.1906x; 1.3028x over previous
"""Trainium2 Bass kernel for batched cosine similarity (retrieval_knn).

sim[s, b] = dot(support[s,b,:], X[b,:]) / (max(||support[s,b]||, eps) * max(||X[b]||, eps))
optionally normalized to (sim + 1) / 2.

Shapes: support [512, 4096, 64] f32, X [4096, 64] f32 -> out [512, 4096] f32.

Strategy (8 NeuronCores, data-parallel over the batch axis):
  - Each core handles a contiguous 512-wide slice of b. 64 MB of support
    data per core; memory-bound problem.
  - Host folds 1/max(||X_b||, eps) into X (Xn), packs Xn into zero-padded
    bf16 matmul weights, so the device only computes:
        dot_n[s,b] = sum_d A[s,b,d] * Xn[b,d]     (TensorE)
        sqn[s,b]   = sum_d A[s,b,d]^2             (TensorE, on squared tiles)
        sim        = dot_n * rsqrt-ish(sqn)       (Scalar/Vector engines)
  - Support is cast fp32->bf16 during the DMA load (SWDGE cast).
  - Natural [s-partition, (b,d)-free] tiles are transposed on TensorE
    (128x128 blocks) into [(b-pair, d)-partition, s-free] layout so the
    d-contraction lands on the partition axis where the PE can reduce it.
  - Per pair of b's: one dot matmul and one squared matmul with N=S,
    accumulated per 32-partition col-group stripe of a PSUM bank.
"""

import numpy as np
import ml_dtypes

BF16 = ml_dtypes.bfloat16

S, B, D = 512, 4096, 64
NCORES = 8
BL = B // NCORES  # 512 batch elements per core

# fraction of PSUM->SBUF evacuation copies routed to ScalarE (rest on VectorE)
_EVAC_MOD = 2  # jq % 3 == 0 -> VectorE, else ScalarE

_PSUM_BUFS = (3, 2, 1, 2)
_NAT_BUFS = 16
_PIPE = 2

_prog_cache = {}


def _build(s_sz, bl_sz, normalize, loop_iters=1, skip=()):
    skip = frozenset(skip)
    from concourse import bacc, mybir
    from concourse.tile import TileContext
    from contextlib import ExitStack, nullcontext

    SBn = s_sz // 128   # number of 128-row s blocks
    Q = bl_sz // 128    # number of 128-wide b quads
    NP = bl_sz // 2     # number of b pairs

    nc = bacc.Bacc("TRN2")
    sup = nc.declare_dram_parameter("support", [s_sz, bl_sz, D], mybir.dt.float32, isOutput=False)
    xw = nc.declare_dram_parameter("xw", [128, NP * 32], mybir.dt.bfloat16, isOutput=False)
    onesw = nc.declare_dram_parameter("onesw", [128, 16 * 32], mybir.dt.bfloat16, isOutput=False)
    idb = nc.declare_dram_parameter("ident_bf", [128, 128], mybir.dt.bfloat16, isOutput=False)
    idf = nc.declare_dram_parameter("ident_f32", [128, 128], mybir.dt.float32, isOutput=False)
    out = nc.declare_dram_parameter("out", [s_sz, bl_sz], mybir.dt.float32, isOutput=True)

    with TileContext(nc) as tc, ExitStack() as ctx:
        singles = ctx.enter_context(tc.tile_pool(name="singles", bufs=1))
        natp = ctx.enter_context(tc.tile_pool(name="nat", bufs=_NAT_BUFS))
        tevp = ctx.enter_context(tc.tile_pool(name="tev", bufs=4))
        sqp = ctx.enter_context(tc.tile_pool(name="sqt", bufs=4))
        finp = ctx.enter_context(tc.tile_pool(name="fin", bufs=2))
        bT, bD, bS, bO = _PSUM_BUFS
        psT = ctx.enter_context(tc.tile_pool(name="psT", bufs=bT, space="PSUM"))
        psDot = ctx.enter_context(tc.tile_pool(name="psDot", bufs=bD, space="PSUM"))
        psSqn = ctx.enter_context(tc.tile_pool(name="psSqn", bufs=bS, space="PSUM"))
        psOut = ctx.enter_context(tc.tile_pool(name="psOut", bufs=bO, space="PSUM"))

        t_idb = singles.tile([128, 128], mybir.dt.bfloat16)
        nc.sync.dma_start(out=t_idb, in_=idb[:, :])
        t_idf = singles.tile([128, 128], mybir.dt.float32)
        nc.sync.dma_start(out=t_idf, in_=idf[:, :])
        t_xw = singles.tile([128, NP * 32], mybir.dt.bfloat16)
        nc.sync.dma_start(out=t_xw, in_=xw[:, :])
        t_ones = singles.tile([128, 16 * 32], mybir.dt.bfloat16)
        nc.sync.dma_start(out=t_ones, in_=onesw[:, :])
        out_stage = [
            singles.tile([128, bl_sz], mybir.dt.float32, name=f"ostage{sb}", tag=f"ostage{sb}")
            for sb in range(SBn)
        ]

        loop_ctx = tc.For_i(0, loop_iters, 1) if loop_iters > 1 else nullcontext()
        ctx.enter_context(loop_ctx)

        ncast_dt = mybir.dt.float32 if "nocast" in skip else mybir.dt.bfloat16
        PIPE = _PIPE  # MMs lag their pair's transposes by this many pairs
        nat_q = {}
        quad_ps = {}
        pair_state = {}

        def load_quad(q):
            nat = {}
            for h in range(2):
                for sb in range(SBn):
                    if "load1" in skip and (h, sb) != (0, 0):
                        nat[(h, sb)] = nat[(0, 0)]
                        continue
                    t = natp.tile([128, 64 * D], ncast_dt, tag="nat", name=f"nat{q}_{h}_{sb}")
                    nc.gpsimd.dma_start(
                        out=t,
                        in_=sup[sb * 128:(sb + 1) * 128,
                                q * 128 + h * 64: q * 128 + (h + 1) * 64, :],
                    )
                    nat[(h, sb)] = t
            nat_q[q] = nat

        def front(jp):
            q, jq = jp // 64, jp % 64
            if jq == 0:
                load_quad(q)
                dot_ps = psDot.tile([128, s_sz], mybir.dt.float32, tag="dotq", name=f"dot{q}")
                sqn_ps = psSqn.tile([128, s_sz], mybir.dt.float32, tag="sqnq", name=f"sqn{q}")
                if "mm" in skip:
                    nc.vector.memset(dot_ps, 0.0)
                    nc.vector.memset(sqn_ps, 1.0)
                quad_ps[q] = (dot_ps, sqn_ps)
            c, l = jq // 16, jq % 16
            h, bh = c // 2, 32 * (c % 2) + 2 * l
            nat = nat_q[q]
            T_ps = None
            if not ("trans" in skip and "evac" in skip):
                T_ps = psT.tile([128, s_sz], mybir.dt.bfloat16, tag="tps", name=f"tps{jp}")
            if "trans" not in skip:
                for sb in range(1 if "trans1" in skip else SBn):
                    nc.tensor.transpose(
                        T_ps[:, sb * 128:(sb + 1) * 128],
                        nat[(h, sb)][:, bh * D:(bh + 2) * D],
                        t_idb,
                    )
            Tt = Sq = None
            if not ("evac" in skip and "mm" in skip):
                Tt = tevp.tile([128, s_sz], mybir.dt.bfloat16, tag="tev", name=f"tt{jp}")
            if "evac" not in skip:
                if jp % _EVAC_MOD == 0:
                    nc.vector.tensor_copy(Tt, T_ps)
                else:
                    nc.scalar.copy(Tt, T_ps)
            if not ("sq" in skip and "mm" in skip):
                Sq = sqp.tile([128, s_sz], mybir.dt.bfloat16, tag="sqt", name=f"sq{jp}")
            if "sq" not in skip:
                nc.vector.tensor_mul(Sq, Tt, Tt)
            pair_state[jp] = (Tt, Sq, c, l)

        def back(jp):
            q, jq = jp // 64, jp % 64
            Tt, Sq, c, l = pair_state.pop(jp)
            dot_ps, sqn_ps = quad_ps[q]
            if "mm" not in skip:
                nc.tensor.matmul(
                    dot_ps[32 * c:32 * (c + 1), :],
                    lhsT=t_xw[:, jp * 32:(jp + 1) * 32],
                    rhs=Tt,
                    start=(l == 0),
                    stop=(l == 15),
                    tile_position=(0, 32 * c),
                )
                nc.tensor.matmul(
                    sqn_ps[32 * c:32 * (c + 1), :],
                    lhsT=t_ones[:, l * 32:(l + 1) * 32],
                    rhs=Sq,
                    start=(l == 0),
                    stop=(l == 15),
                    tile_position=(0, 32 * c),
                )
            if jq == 63:
                finalize(q)

        def finalize(q):
            dot_ps, sqn_ps = quad_ps.pop(q)
            sqv = finp.tile([128, s_sz], mybir.dt.float32, tag="fsq", name=f"fsq{q}")
            nc.scalar.sqrt(sqv, sqn_ps)
            nc.vector.tensor_scalar_max(sqv, sqv, 1e-10)
            rv = finp.tile([128, s_sz], mybir.dt.float32, tag="frv", name=f"frv{q}")
            nc.vector.reciprocal(rv, sqv)
            simv = finp.tile([128, s_sz], mybir.dt.float32, tag="fsim", name=f"fsim{q}")
            nc.vector.tensor_mul(simv, dot_ps, rv)
            if normalize:
                nc.vector.tensor_scalar(
                    simv, simv, 0.5, 0.5, mybir.AluOpType.mult, mybir.AluOpType.add
                )
            for sb in range(SBn):
                oT = psOut.tile([128, 128], mybir.dt.float32, tag="ot", name=f"ot{q}{sb}")
                nc.tensor.transpose(oT, simv[:, sb * 128:(sb + 1) * 128], t_idf)
                nc.vector.tensor_copy(out_stage[sb][:, q * 128:(q + 1) * 128], oT)
                nc.sync.dma_start(
                    out=out[sb * 128:(sb + 1) * 128, q * 128:(q + 1) * 128],
                    in_=out_stage[sb][:, q * 128:(q + 1) * 128],
                )

        for jp in range(NP + PIPE):
            if jp < NP:
                front(jp)
            if jp - PIPE >= 0:
                back(jp - PIPE)

    nc.finalize()
    return nc


def _pack_host_inputs(x_hat, bl_sz):
    """Fold 1/max(||x||,eps) into X, pack per-core zero-padded bf16 lhsT mats.

    Returns (xw_per_core list of [128, (bl//2)*32] bf16, onesw [128, 512] bf16).
    Pair jp (within a core) covers b_local = q*128 + 32*c + 2*l (+1), where
    q = jp // 64, c = (jp % 64) // 16, l = jp % 16.  lhsT column 2*l holds
    Xn[b_even] in partitions 0:64, column 2*l+1 holds Xn[b_odd] in 64:128.
    """
    x = np.asarray(x_hat, np.float32)
    xnorm = np.sqrt((x * x).sum(axis=1, keepdims=True))
    xn = (x / np.maximum(xnorm, 1e-10)).astype(BF16)

    ncores = x.shape[0] // bl_sz
    np_pairs = bl_sz // 2
    xw_cores = []
    for k in range(ncores):
        xw = np.zeros((128, np_pairs * 32), dtype=BF16)
        for jp in range(np_pairs):
            q, jq = jp // 64, jp % 64
            c, l = jq // 16, jq % 16
            b0 = k * bl_sz + q * 128 + 32 * c + 2 * l
            col = jp * 32
            xw[0:64, col + 2 * l] = xn[b0]
            xw[64:128, col + 2 * l + 1] = xn[b0 + 1]
        xw_cores.append(xw)

    onesw = np.zeros((128, 16 * 32), dtype=BF16)
    for l in range(16):
        onesw[0:64, l * 32 + 2 * l] = BF16(1.0)
        onesw[64:128, l * 32 + 2 * l + 1] = BF16(1.0)
    return xw_cores, onesw


def _get_program(normalize):
    key = (S, BL, bool(normalize))
    if key not in _prog_cache:
        _prog_cache[key] = _build(S, BL, bool(normalize))
    return _prog_cache[key]


def _run(support_set, X_hat, normalize, **spmd_kwargs):
    support_set = np.asarray(support_set)
    X_hat = np.asarray(X_hat, np.float32)
    nrm = bool(np.asarray(normalize).item())

    from concourse.bass_utils import run_bass_kernel_spmd

    nc = _get_program(nrm)
    xw_cores, onesw = _pack_host_inputs(X_hat, BL)
    ident_bf = np.eye(128, dtype=BF16)
    ident_f32 = np.eye(128, dtype=np.float32)

    in_maps = []
    for k in range(NCORES):
        shard = np.ascontiguousarray(support_set[:, k * BL:(k + 1) * BL, :], dtype=np.float32)
        in_maps.append({
            "support": shard,
            "xw": xw_cores[k],
            "onesw": onesw,
            "ident_bf": ident_bf,
            "ident_f32": ident_f32,
        })

    res = run_bass_kernel_spmd(nc, in_maps, list(range(NCORES)), **spmd_kwargs)
    out = np.concatenate(
        [np.asarray(res.results[k]["out"]) for k in range(NCORES)], axis=1
    )
    return np.ascontiguousarray(out, dtype=np.float32), res


def kernel(support_set, X_hat, normalize):
    out, _ = _run(support_set, X_hat, normalize)
    return out


# revision 18
# speedup vs baseline: 186.7137x; 2.5167x over previous
"""Trainium2 Bass kernel for batched cosine similarity (retrieval_knn).

sim[s, b] = dot(support[s,b,:], X[b,:]) / (max(||support[s,b]||, eps) * max(||X[b]||, eps))
optionally normalized to (sim + 1) / 2.

Shapes: support [512, 4096, 64] f32, X [4096, 64] f32 -> out [512, 4096] f32.

Strategy (8 NeuronCores, data-parallel over the batch axis):
  - Each core handles a contiguous 512-wide slice of b. 64 MB of support
    data per core; memory-bound problem.
  - Host folds 1/max(||X_b||, eps) into X (Xn), packs Xn into zero-padded
    bf16 matmul weights, so the device only computes:
        dot_n[s,b] = sum_d A[s,b,d] * Xn[b,d]     (TensorE)
        sqn[s,b]   = sum_d A[s,b,d]^2             (TensorE, on squared tiles)
        sim        = dot_n * rsqrt-ish(sqn)       (Scalar/Vector engines)
  - Support is cast fp32->bf16 during the DMA load (SWDGE cast).
  - Natural [s-partition, (b,d)-free] tiles are transposed on TensorE
    (128x128 blocks) into [(b-pair, d)-partition, s-free] layout so the
    d-contraction lands on the partition axis where the PE can reduce it.
  - Per pair of b's: one dot matmul and one squared matmul with N=S,
    accumulated per 32-partition col-group stripe of a PSUM bank.
"""

import numpy as np
import ml_dtypes

BF16 = ml_dtypes.bfloat16

S, B, D = 512, 4096, 64
NCORES = 8
BL = B // NCORES  # 512 batch elements per core

# fraction of PSUM->SBUF evacuation copies routed to ScalarE (rest on VectorE)
_EVAC_MOD = 2  # jq % 3 == 0 -> VectorE, else ScalarE

_PSUM_BUFS = (3, 2, 1, 2)
_TEV_BUFS = 4
_NAT_BUFS = 16
_PIPE = 0

_prog_cache = {}


def _build(s_sz, bl_sz, normalize, loop_iters=1, skip=()):
    skip = frozenset(skip)
    from concourse import bacc, mybir
    from concourse.tile import TileContext
    from contextlib import ExitStack, nullcontext

    SBn = s_sz // 128   # number of 128-row s blocks
    Q = bl_sz // 128    # number of 128-wide b quads
    NP = bl_sz // 2     # number of b pairs

    nc = bacc.Bacc("TRN2")
    sup = nc.declare_dram_parameter("support", [s_sz, bl_sz, D], mybir.dt.float32, isOutput=False)
    xw = nc.declare_dram_parameter("xw", [128, NP * 32], mybir.dt.bfloat16, isOutput=False)
    onesw = nc.declare_dram_parameter("onesw", [128, 16 * 32], mybir.dt.bfloat16, isOutput=False)
    idb = nc.declare_dram_parameter("ident_bf", [128, 128], mybir.dt.bfloat16, isOutput=False)
    idf = nc.declare_dram_parameter("ident_f32", [128, 128], mybir.dt.float32, isOutput=False)
    out = nc.declare_dram_parameter("out", [s_sz, bl_sz], mybir.dt.float32, isOutput=True)

    with TileContext(nc) as tc, ExitStack() as ctx:
        singles = ctx.enter_context(tc.tile_pool(name="singles", bufs=1))
        natp = ctx.enter_context(tc.tile_pool(name="nat", bufs=_NAT_BUFS))
        tevp = ctx.enter_context(tc.tile_pool(name="tev", bufs=_TEV_BUFS))
        sqp = ctx.enter_context(tc.tile_pool(name="sqt", bufs=_TEV_BUFS))
        finp = ctx.enter_context(tc.tile_pool(name="fin", bufs=2))
        bT, bD, bS, bO = _PSUM_BUFS
        psT = ctx.enter_context(tc.tile_pool(name="psT", bufs=bT, space="PSUM"))
        psDot = ctx.enter_context(tc.tile_pool(name="psDot", bufs=bD, space="PSUM"))
        psSqn = ctx.enter_context(tc.tile_pool(name="psSqn", bufs=bS, space="PSUM"))
        psOut = ctx.enter_context(tc.tile_pool(name="psOut", bufs=bO, space="PSUM"))

        t_idb = singles.tile([128, 128], mybir.dt.bfloat16)
        nc.sync.dma_start(out=t_idb, in_=idb[:, :])
        t_idf = singles.tile([128, 128], mybir.dt.float32)
        nc.sync.dma_start(out=t_idf, in_=idf[:, :])
        t_xw = singles.tile([128, NP * 32], mybir.dt.bfloat16)
        nc.sync.dma_start(out=t_xw, in_=xw[:, :])
        t_ones = singles.tile([128, 16 * 32], mybir.dt.bfloat16)
        nc.sync.dma_start(out=t_ones, in_=onesw[:, :])
        out_stage = [
            singles.tile([128, bl_sz], mybir.dt.float32, name=f"ostage{sb}", tag=f"ostage{sb}")
            for sb in range(SBn)
        ]

        loop_ctx = tc.For_i(0, loop_iters, 1) if loop_iters > 1 else nullcontext()
        ctx.enter_context(loop_ctx)

        ncast_dt = mybir.dt.float32 if "nocast" in skip else mybir.dt.bfloat16
        PIPE = _PIPE  # MMs lag their pair's transposes by this many pairs
        nat_q = {}
        quad_ps = {}
        pair_state = {}

        def load_quad(q):
            nat = {}
            for h in range(2):
                for sb in range(SBn):
                    if "load1" in skip and (h, sb) != (0, 0):
                        nat[(h, sb)] = nat[(0, 0)]
                        continue
                    t = natp.tile([128, 64 * D], ncast_dt, tag="nat", name=f"nat{q}_{h}_{sb}")
                    nc.gpsimd.dma_start(
                        out=t,
                        in_=sup[sb * 128:(sb + 1) * 128,
                                q * 128 + h * 64: q * 128 + (h + 1) * 64, :],
                    )
                    nat[(h, sb)] = t
            nat_q[q] = nat

        def front(jp):
            q, jq = jp // 64, jp % 64
            if jq == 0:
                load_quad(q)
                dot_ps = psDot.tile([128, s_sz], mybir.dt.float32, tag="dotq", name=f"dot{q}")
                sqn_ps = psSqn.tile([128, s_sz], mybir.dt.float32, tag="sqnq", name=f"sqn{q}")
                if "mm" in skip:
                    nc.vector.memset(dot_ps, 0.0)
                    nc.vector.memset(sqn_ps, 1.0)
                quad_ps[q] = (dot_ps, sqn_ps)
            c, l = jq // 16, jq % 16
            h, bh = c // 2, 32 * (c % 2) + 2 * l
            nat = nat_q[q]
            T_ps = None
            if not ("trans" in skip and "evac" in skip):
                T_ps = psT.tile([128, s_sz], mybir.dt.bfloat16, tag="tps", name=f"tps{jp}")
            if "trans" not in skip:
                for sb in range(1 if "trans1" in skip else SBn):
                    nc.tensor.transpose(
                        T_ps[:, sb * 128:(sb + 1) * 128],
                        nat[(h, sb)][:, bh * D:(bh + 2) * D],
                        t_idb,
                    )
            Tt = Sq = None
            if not ("evac" in skip and "mm" in skip):
                Tt = tevp.tile([128, s_sz], mybir.dt.bfloat16, tag="tev", name=f"tt{jp}")
            if "evac" not in skip:
                if jp % _EVAC_MOD == 0:
                    nc.vector.tensor_copy(Tt, T_ps)
                else:
                    nc.scalar.copy(Tt, T_ps)
            if not ("sq" in skip and "mm" in skip):
                Sq = sqp.tile([128, s_sz], mybir.dt.bfloat16, tag="sqt", name=f"sq{jp}")
            if "sq" not in skip:
                nc.vector.tensor_mul(Sq, Tt, Tt)
            pair_state[jp] = (Tt, Sq, c, l)

        def back(jp):
            q, jq = jp // 64, jp % 64
            Tt, Sq, c, l = pair_state.pop(jp)
            dot_ps, sqn_ps = quad_ps[q]
            if "mm" not in skip:
                nc.tensor.matmul(
                    dot_ps[32 * c:32 * (c + 1), :],
                    lhsT=t_xw[:, jp * 32:(jp + 1) * 32],
                    rhs=Tt,
                    start=(l == 0),
                    stop=(l == 15),
                    tile_position=(0, 32 * c),
                )
                nc.tensor.matmul(
                    sqn_ps[32 * c:32 * (c + 1), :],
                    lhsT=t_ones[:, l * 32:(l + 1) * 32],
                    rhs=Sq,
                    start=(l == 0),
                    stop=(l == 15),
                    tile_position=(0, 32 * c),
                )
            if jq == 63:
                finalize(q)

        def finalize(q):
            dot_ps, sqn_ps = quad_ps.pop(q)
            sqv = finp.tile([128, s_sz], mybir.dt.float32, tag="fsq", name=f"fsq{q}")
            nc.scalar.sqrt(sqv, sqn_ps)
            nc.vector.tensor_scalar_max(sqv, sqv, 1e-10)
            rv = finp.tile([128, s_sz], mybir.dt.float32, tag="frv", name=f"frv{q}")
            nc.vector.reciprocal(rv, sqv)
            simv = finp.tile([128, s_sz], mybir.dt.float32, tag="fsim", name=f"fsim{q}")
            nc.vector.tensor_mul(simv, dot_ps, rv)
            if normalize:
                nc.vector.tensor_scalar(
                    simv, simv, 0.5, 0.5, mybir.AluOpType.mult, mybir.AluOpType.add
                )
            for sb in range(SBn):
                oT = psOut.tile([128, 128], mybir.dt.float32, tag="ot", name=f"ot{q}{sb}")
                nc.tensor.transpose(oT, simv[:, sb * 128:(sb + 1) * 128], t_idf)
                nc.vector.tensor_copy(out_stage[sb][:, q * 128:(q + 1) * 128], oT)
                nc.sync.dma_start(
                    out=out[sb * 128:(sb + 1) * 128, q * 128:(q + 1) * 128],
                    in_=out_stage[sb][:, q * 128:(q + 1) * 128],
                )

        for jp in range(NP + PIPE):
            if jp < NP:
                front(jp)
            if jp - PIPE >= 0:
                back(jp - PIPE)

    nc.finalize()
    return nc


def _pack_host_inputs(x_hat, bl_sz):
    """Fold 1/max(||x||,eps) into X, pack per-core zero-padded bf16 lhsT mats.

    Returns (xw_per_core list of [128, (bl//2)*32] bf16, onesw [128, 512] bf16).
    Pair jp (within a core) covers b_local = q*128 + 32*c + 2*l (+1), where
    q = jp // 64, c = (jp % 64) // 16, l = jp % 16.  lhsT column 2*l holds
    Xn[b_even] in partitions 0:64, column 2*l+1 holds Xn[b_odd] in 64:128.
    """
    x = np.asarray(x_hat, np.float32)
    xnorm = np.sqrt((x * x).sum(axis=1, keepdims=True))
    xn = (x / np.maximum(xnorm, 1e-10)).astype(BF16)

    ncores = x.shape[0] // bl_sz
    np_pairs = bl_sz // 2
    xw_cores = []
    for k in range(ncores):
        xw = np.zeros((128, np_pairs * 32), dtype=BF16)
        for jp in range(np_pairs):
            q, jq = jp // 64, jp % 64
            c, l = jq // 16, jq % 16
            b0 = k * bl_sz + q * 128 + 32 * c + 2 * l
            col = jp * 32
            xw[0:64, col + 2 * l] = xn[b0]
            xw[64:128, col + 2 * l + 1] = xn[b0 + 1]
        xw_cores.append(xw)

    onesw = np.zeros((128, 16 * 32), dtype=BF16)
    for l in range(16):
        onesw[0:64, l * 32 + 2 * l] = BF16(1.0)
        onesw[64:128, l * 32 + 2 * l + 1] = BF16(1.0)
    return xw_cores, onesw


def _get_program(normalize):
    key = (S, BL, bool(normalize))
    if key not in _prog_cache:
        _prog_cache[key] = _build(S, BL, bool(normalize))
    return _prog_cache[key]


def _run(support_set, X_hat, normalize, **spmd_kwargs):
    support_set = np.asarray(support_set)
    X_hat = np.asarray(X_hat, np.float32)
    nrm = bool(np.asarray(normalize).item())

    from concourse.bass_utils import run_bass_kernel_spmd

    nc = _get_program(nrm)
    xw_cores, onesw = _pack_host_inputs(X_hat, BL)
    ident_bf = np.eye(128, dtype=BF16)
    ident_f32 = np.eye(128, dtype=np.float32)

    in_maps = []
    for k in range(NCORES):
        shard = np.ascontiguousarray(support_set[:, k * BL:(k + 1) * BL, :], dtype=np.float32)
        in_maps.append({
            "support": shard,
            "xw": xw_cores[k],
            "onesw": onesw,
            "ident_bf": ident_bf,
            "ident_f32": ident_f32,
        })

    res = run_bass_kernel_spmd(nc, in_maps, list(range(NCORES)), **spmd_kwargs)
    out = np.concatenate(
        [np.asarray(res.results[k]["out"]) for k in range(NCORES)], axis=1
    )
    return np.ascontiguousarray(out, dtype=np.float32), res


def kernel(support_set, X_hat, normalize):
    out, _ = _run(support_set, X_hat, normalize)
    return out
